# revision 17
# baseline (speedup 1.0000x reference)
"""3-layer GAT (nn_GAT_7095285973830) on 8 Trainium2 NeuronCores.

Strategy
--------
Host: relabel nodes into 64-node "windows" bin-packed so each window has
<= KP*128 incoming edges; sort edges by destination window; 8 cores each
own a contiguous block of windows (== contiguous node-slot range).

Device (per layer): for each window, indirect-DMA-gather table rows for
the window's edge sources ([features | 1 | el | er] packed rows), compute
edge scores ex = exp(leakyrelu(el_src + er_dst)) (no segment-max needed:
scores are small), build per-head ex-weighted one-hot matrices over the
64 destination slots (head pairs stacked to 128 matmul columns), and let
the TensorEngine do scaling + segment-sum + softmax denominators in one
PSUM accumulation (denominator = the gathered "1" column).  Flush divides
by the denominator, applies the layer's transform/bias/activation, and
writes next-layer table rows.  Between layers, AllGather replicates the
per-core table slices.

Layer 0 and 2 aggregate *input* features then transform (cheaper gathers),
layer 1 aggregates pre-transformed per-head features.
"""

import numpy as np

# ---------------- problem constants (hardcoded per contract) ----------------
N_NODES = 50000
N_EDGES = 800000
IN_F = 128
HID = 64
HEADS = 4
N_CLASSES = 256
NEG = 0.2

P = 128          # partitions / edge-tile size
WN = 64          # nodes per window
NCORE = 8

# table row layouts (f32 columns)
FW0 = 144        # L0: feat 0:128 | 1 @128 | el 129:133 | er 133:137 | pad
FW12 = 272       # L1: z01 0:128 | 1 @128 | z23 129:257 | 1 @257 | el 258:262 | er 262:266 | pad
                 # L2: h2 0:256  | 1 @256 | el 257:261 | er 261:265 | pad

_CACHE = {}


def _bin_pack(deg, nwin):
    """Assign nodes to nwin bins of <=WN nodes, balancing summed degree."""
    import heapq
    order = np.argsort(-deg, kind="stable")
    heap = [(0, 0, w) for w in range(nwin)]
    heapq.heapify(heap)
    assign = np.empty(len(deg), np.int32)
    for n in order:
        while True:
            s, c, w = heapq.heappop(heap)
            if c < WN:
                break
        assign[n] = w
        heapq.heappush(heap, (s + int(deg[n]), c + 1, w))
    return assign


def _preprocess(src, dst, n_nodes):
    """Node relabeling + edge packing. Returns dict of host-side structures."""
    deg = np.bincount(dst, minlength=n_nodes)
    nwin = NCORE * int(np.ceil(n_nodes / (WN * NCORE) * 1.0344))  # ~3.4% slot slack
    nwin = max(nwin, NCORE)
    assign = _bin_pack(deg, nwin)

    # slots: nodes of window w occupy slots w*WN + [0..)
    order = np.argsort(assign, kind="stable")
    nslot = nwin * WN
    slot_of = np.full(n_nodes, -1, np.int64)
    slot2node = np.full(nslot, -1, np.int64)
    win_counts = np.bincount(assign, minlength=nwin)
    base = np.repeat(np.arange(nwin) * WN, win_counts)
    pos_in_win = np.arange(len(order)) - np.repeat(
        np.concatenate([[0], np.cumsum(win_counts)[:-1]]), win_counts
    )
    slots = base + pos_in_win
    slot_of[order] = slots
    slot2node[slots] = order

    s_slot = slot_of[src]
    d_slot = slot_of[dst]
    w_e = (d_slot // WN).astype(np.int64)
    ecnt = np.bincount(w_e, minlength=nwin)
    kp = max(8, int(np.ceil(ecnt.max() / P)))
    cap = kp * P

    eorder = np.argsort(w_e, kind="stable")
    offs = np.concatenate([[0], np.cumsum(ecnt)[:-1]])
    pos = np.arange(len(eorder)) - np.repeat(offs, ecnt)
    srcA = np.zeros((nwin, cap), np.int32)
    dstB = np.zeros((nwin, cap), np.int32)
    dloc = np.full((nwin, cap), -1.0, np.float32)
    we_s = w_e[eorder]
    srcA[we_s, pos] = s_slot[eorder]
    dstB[we_s, pos] = d_slot[eorder]
    dloc[we_s, pos] = (d_slot[eorder] % WN).astype(np.float32)

    # [nwin, cap] -> [nwin, P, kp] with edge (w, t, p) = packed index t*P+p
    srcA = srcA.reshape(nwin, kp, P).transpose(0, 2, 1).copy()
    dstB = dstB.reshape(nwin, kp, P).transpose(0, 2, 1).copy()
    dloc = dloc.reshape(nwin, kp, P).transpose(0, 2, 1).copy()
    return dict(
        nwin=nwin, nslot=nslot, kp=kp, slot_of=slot_of, slot2node=slot2node,
        srcA=srcA, dstB=dstB, dloc=dloc,
    )


def _build(pp, phases=3):
    """Build + compile the Bass program for the given packing.
    phases: 1 = layer0 only, 2 = layers 0-1, 3 = full."""
    import concourse.bacc as bacc
    import concourse.mybir as mybir
    import concourse.tile as tile
    from concourse.bass import AP, IndirectOffsetOnAxis
    from concourse.masks import make_identity

    DT = mybir.dt.float32
    I32 = mybir.dt.int32
    AF = mybir.ActivationFunctionType
    ALU = mybir.AluOpType

    KP = pp["kp"]
    NWIN = pp["nwin"]
    NSLOT = pp["nslot"]
    NWIN_C = NWIN // NCORE
    SLOT_C = NSLOT // NCORE

    nc = bacc.Bacc("TRN2", num_devices=NCORE)

    # ---- I/O ----
    ext0_d = nc.dram_tensor("ext0", [NSLOT, FW0], DT, kind="ExternalInput")
    wi_d = nc.dram_tensor("windat_i", [NWIN_C, P, 2 * KP], I32, kind="ExternalInput")
    wf_d = nc.dram_tensor("windat_f", [NWIN_C, P, KP], DT, kind="ExternalInput")
    W0_d = nc.dram_tensor("W0big", [HEADS, IN_F, 256], DT, kind="ExternalInput")
    W1_d = nc.dram_tensor("W1", [256, 256], DT, kind="ExternalInput")
    W2_d = nc.dram_tensor("W2", [256, 1024], DT, kind="ExternalInput")
    wlr1_d = nc.dram_tensor("wlr1", [256, 8], DT, kind="ExternalInput")
    wlr2_d = nc.dram_tensor("wlr2", [256, 8], DT, kind="ExternalInput")
    b0_d = nc.dram_tensor("b0row", [1, 256], DT, kind="ExternalInput")
    b1s_d = nc.dram_tensor("b1s", [2, P, P], DT, kind="ExternalInput")
    b2m_d = nc.dram_tensor("b2mean", [1, 256], DT, kind="ExternalInput")
    logits_d = nc.dram_tensor("logits", [SLOT_C, 256], DT, kind="ExternalOutput")

    z1_slice = nc.dram_tensor("z1_slice", [SLOT_C, FW12], DT,
                              kind="ExternalOutput" if phases == 1 else "Internal")
    _as = "Local" if __import__("os").environ.get("GAT_NOSHARED") else "Shared"
    z1_full = nc.dram_tensor("z1_full", [NSLOT, FW12], DT, addr_space=_as)
    h2_slice = nc.dram_tensor("h2_slice", [SLOT_C, FW12], DT,
                              kind="ExternalOutput" if phases == 2 else "Internal")
    h2_full = nc.dram_tensor("h2_full", [NSLOT, FW12], DT, addr_space=_as)

    RG = [list(range(NCORE))]

    with tile.TileContext(nc) as tc:
        with (
            tc.tile_pool(name="const", bufs=1) as cp,
            tc.tile_pool(name="gath", bufs=3) as gp,
            tc.tile_pool(name="work", bufs=3) as wp,
            tc.tile_pool(name="flush", bufs=2) as fp,
            tc.tile_pool(name="ps", bufs=2, space="PSUM") as ps,
            tc.tile_pool(name="pst", bufs=2, space="PSUM") as pst,
            # PSUM budget: 8 banks total. ps: agg0/agg1 x2 bufs = 4 banks.
            # pst: "tp" (transposes) x2 + "acc" (transform accums) x2 = 4.
        ):
            # ---- constants ----
            ident = cp.tile([P, P], DT)
            make_identity(nc, ident[:])
            iota_i = cp.tile([P, KP * WN], I32)
            nc.gpsimd.iota(iota_i[:], pattern=[[0, KP], [1, WN]], channel_multiplier=0)
            iota_f = cp.tile([P, KP * WN], DT)
            nc.vector.tensor_copy(iota_f[:], iota_i[:])
            ones1 = cp.tile([1, WN], DT)
            nc.vector.memset(ones1[:], 1.0)

            W0sb = [cp.tile([IN_F, 256], DT, tag=f"W0_{h}", name=f"W0_{h}")
                    for h in range(HEADS)]
            for h in range(HEADS):
                nc.sync.dma_start(out=W0sb[h][:], in_=W0_d[h])
            W1sb = [cp.tile([P, 256], DT, tag=f"W1_{i}", name=f"W1_{i}") for i in range(2)]
            W2sb = [cp.tile([P, 1024], DT, tag=f"W2_{i}", name=f"W2_{i}") for i in range(2)]
            wlr1sb = [cp.tile([P, 8], DT, tag=f"wlr1_{i}", name=f"wlr1_{i}") for i in range(2)]
            wlr2sb = [cp.tile([P, 8], DT, tag=f"wlr2_{i}", name=f"wlr2_{i}") for i in range(2)]
            for i in range(2):
                nc.sync.dma_start(out=W1sb[i][:], in_=W1_d[i * P : (i + 1) * P, :])
                nc.sync.dma_start(out=W2sb[i][:], in_=W2_d[i * P : (i + 1) * P, :])
                nc.sync.dma_start(out=wlr1sb[i][:], in_=wlr1_d[i * P : (i + 1) * P, :])
                nc.sync.dma_start(out=wlr2sb[i][:], in_=wlr2_d[i * P : (i + 1) * P, :])
            b0sb = cp.tile([1, 256], DT)
            nc.sync.dma_start(out=b0sb[:], in_=b0_d[:, :])
            b2sb = cp.tile([1, 256], DT)
            nc.sync.dma_start(out=b2sb[:], in_=b2m_d[:, :])
            b1s01 = cp.tile([P, P], DT)
            b1s23 = cp.tile([P, P], DT)
            nc.sync.dma_start(out=b1s01[:], in_=b1s_d[0])
            nc.sync.dma_start(out=b1s23[:], in_=b1s_d[1])

            def elu(dst_ap, src_ap, pp_, tag):
                """dst = elu(src). src may be PSUM; dst SBUF."""
                shape = [src_ap.shape[0], src_ap.free_size()]
                r = wp.tile(shape, DT, tag=f"elu_r{tag}")
                m = wp.tile(shape, DT, tag=f"elu_m{tag}")
                nc.scalar.activation(out=r[:], in_=src_ap, func=AF.Relu)
                nc.vector.tensor_scalar(out=m[:], in0=src_ap, scalar1=0.0,
                                        scalar2=None, op0=ALU.min)
                nc.scalar.activation(out=m[:], in_=m[:], func=AF.Exp)
                nc.vector.scalar_tensor_tensor(out=dst_ap, in0=m[:], scalar=-1.0,
                                               in1=r[:], op0=ALU.add, op1=ALU.add)

            def layer(lidx, table_d, fw, rhs_cols, el_lo, er_off, nmsg, flush):
                """Emit one GAT layer's window loop.
                rhs_cols: [(lo, hi)] for head-pair 0/1 matmul rhs slices.
                el_lo: start col of el in gathered row. er_off: element offset
                of er in table row. nmsg: rhs width (incl. den col)."""
                for w in range(NWIN_C):
                    wi = wp.tile([P, 2 * KP], I32, tag="wi")
                    dl = wp.tile([P, KP], DT, tag="dl")
                    nc.sync.dma_start(out=wi[:], in_=wi_d[w])
                    nc.sync.dma_start(out=dl[:], in_=wf_d[w])

                    gath = gp.tile([P, KP * fw], DT, tag="gath")
                    gA = gath[:].rearrange("p (k c) -> p k c", k=KP)
                    gerb = gp.tile([P, KP * 4], DT, tag="gerb")
                    gB = gerb[:].rearrange("p (k c) -> p k c", k=KP)
                    for t in range(KP):
                        nc.gpsimd.indirect_dma_start(
                            out=gA[:, t, :], out_offset=None,
                            in_=table_d[:, :],
                            in_offset=IndirectOffsetOnAxis(
                                ap=wi[:, t : t + 1], axis=0),
                        )
                        nc.gpsimd.indirect_dma_start(
                            out=gB[:, t, :], out_offset=None,
                            in_=table_d[:, :],
                            in_offset=IndirectOffsetOnAxis(
                                ap=wi[:, KP + t : KP + t + 1], axis=0),
                            element_offset=er_off,
                        )

                    # ex = exp(lrelu(el_src + er_dst))
                    ex = wp.tile([P, KP * 4], DT, tag="ex")
                    exr = ex[:].rearrange("p (k h) -> p k h", k=KP)
                    nc.vector.tensor_tensor(out=exr, in0=gA[:, :, el_lo : el_lo + 4],
                                            in1=gB, op=ALU.add)
                    nc.vector.scalar_tensor_tensor(out=exr, in0=exr, scalar=NEG,
                                                   in1=exr, op0=ALU.mult, op1=ALU.max)
                    nc.scalar.activation(out=ex[:], in_=ex[:], func=AF.Exp)

                    # one-hot over window slots
                    oh = wp.tile([P, KP * WN], DT, tag="oh")
                    ohr = oh[:].rearrange("p (k w) -> p k w", k=KP)
                    dl_b = AP(dl[:].tensor, dl[:].offset,
                              [dl[:].ap[0], [1, KP], [0, WN]])
                    nc.vector.tensor_tensor(
                        out=ohr, in0=dl_b,
                        in1=iota_f[:].rearrange("p (k w) -> p k w", k=KP),
                        op=ALU.is_equal)

                    # ex-weighted one-hots, head pairs stacked on columns
                    Obufs = []
                    for hp in range(2):
                        O = wp.tile([P, KP * P], DT, tag=f"O{hp}")
                        Or = O[:].rearrange("p (k c) -> p k c", k=KP)
                        for hh in range(2):
                            h = 2 * hp + hh
                            nc.vector.tensor_tensor(
                                out=Or[:, :, hh * WN : (hh + 1) * WN], in0=ohr,
                                in1=exr[:, :, h : h + 1].to_broadcast([P, KP, WN]),
                                op=ALU.mult)
                        Obufs.append(Or)

                    # aggregation matmuls (msg + den accumulate in PSUM)
                    psx = []
                    for hp in range(2):
                        pt = ps.tile([P, nmsg], DT, tag=f"agg{hp}")
                        lo, hi = rhs_cols[hp]
                        for t in range(KP):
                            nc.tensor.matmul(pt[:], lhsT=Obufs[hp][:, t, :],
                                             rhs=gA[:, t, lo:hi],
                                             start=(t == 0), stop=(t == KP - 1))
                        psx.append(pt)

                    # normalize by denominator column
                    norms = []
                    for hp in range(2):
                        rden = wp.tile([P, 1], DT, tag=f"rden{hp}")
                        nc.vector.tensor_scalar(out=rden[:],
                                                in0=psx[hp][:, nmsg - 1 : nmsg],
                                                scalar1=1e-30, scalar2=None,
                                                op0=ALU.max)
                        nc.vector.reciprocal(rden[:], rden[:])
                        nm = fp.tile([P, nmsg - 1], DT, tag=f"norm{hp}")
                        nc.vector.tensor_scalar(out=nm[:], in0=psx[hp][:, 0 : nmsg - 1],
                                                scalar1=rden[:], scalar2=None,
                                                op0=ALU.mult)
                        norms.append(nm)
                    flush(w, norms)

            # ---------------- layer 0 flush ----------------
            def flush0(w, norms):
                # transpose stacked agg -> [k, nodes]
                aT = []
                for hp in range(2):
                    tp = pst.tile([P, P], DT, tag="tp")
                    nc.tensor.transpose(out=tp[:], in_=norms[hp][:], identity=ident[:])
                    a = fp.tile([P, P], DT, tag=f"aT{hp}")
                    nc.vector.tensor_copy(a[:], tp[:])
                    aT.append(a)
                # h1 = elu(agg @ W0 + b0)
                pt = pst.tile([WN, 256], DT, tag="acc")
                for h in range(4):
                    nc.tensor.matmul(
                        pt[:],
                        lhsT=aT[h // 2][:, (h % 2) * WN : (h % 2 + 1) * WN],
                        rhs=W0sb[h][:],
                        start=(h == 0), stop=False)
                nc.tensor.matmul(pt[:], lhsT=ones1[:], rhs=b0sb[:],
                                 start=False, stop=True)
                h1 = fp.tile([WN, 256], DT, tag="h1")
                elu(h1[:], pt[:], fp, "0")
                # z1 = h1 @ W1, el/er = h1 @ wlr1
                hT = []
                for i in range(2):
                    tp2 = pst.tile([P, P], DT, tag="tp")
                    nc.tensor.transpose(out=tp2[:, 0:WN],
                                        in_=h1[:, i * P : (i + 1) * P],
                                        identity=ident[0:WN, 0:WN])
                    hh = fp.tile([P, WN], DT, tag=f"hT{i}")
                    nc.vector.tensor_copy(hh[:], tp2[:, 0:WN])
                    hT.append(hh)
                zp = pst.tile([WN, 256], DT, tag="acc")
                sp = pst.tile([WN, 8], DT, tag="acc")
                for i in range(2):
                    nc.tensor.matmul(zp[:], lhsT=hT[i][:], rhs=W1sb[i][:],
                                     start=(i == 0), stop=(i == 1))
                    nc.tensor.matmul(sp[:], lhsT=hT[i][:], rhs=wlr1sb[i][:],
                                     start=(i == 0), stop=(i == 1))
                row = fp.tile([WN, FW12], DT, tag="row")
                nc.vector.tensor_copy(row[:, 0:128], zp[:, 0:128])
                nc.vector.tensor_copy(row[:, 129:257], zp[:, 128:256])
                nc.vector.tensor_copy(row[:, 258:266], sp[:])
                nc.vector.memset(row[:, 128:129], 1.0)
                nc.vector.memset(row[:, 257:258], 1.0)
                nc.vector.memset(row[:, 266:FW12], 0.0)
                nc.sync.dma_start(out=z1_slice[w * WN : (w + 1) * WN, :], in_=row[:])

            # ---------------- layer 1 flush ----------------
            def flush1_stub(w, norms):
                rows = h2_slice[w * WN : (w + 1) * WN, :]
                tmp = fp.tile([P, FW12], DT, tag="stub")
                nc.vector.memset(tmp[:], 0.0)
                nc.vector.tensor_copy(tmp[:, 0:128], norms[0][:])
                nc.vector.tensor_copy(tmp[:, 128:256], norms[1][:])
                nc.sync.dma_start(out=rows[:, :], in_=tmp[0:WN, :])

            def flush1(w, norms):
                import os as _os
                cut = _os.environ.get("GAT_F1CUT", "")
                rows = h2_slice[w * WN : (w + 1) * WN, :]
                sT = []
                for hp in range(2):
                    e = fp.tile([P, P], DT, tag=f"eb{hp}")
                    nc.vector.tensor_tensor(out=e[:], in0=norms[hp][:],
                                            in1=(b1s01 if hp == 0 else b1s23)[:],
                                            op=ALU.add)
                    el_t = fp.tile([P, P], DT, tag=f"el{hp}")
                    elu(el_t[:], e[:], fp, "1")
                    # write h2 quadrants straight to DRAM (L2 row layout:
                    # h2 0:256 | 1 @256 | el 257:261 | er 261:265 | pad)
                    nc.sync.dma_start(
                        out=rows[:, 2 * hp * WN : (2 * hp + 1) * WN],
                        in_=el_t[0:WN, 0:WN])
                    nc.sync.dma_start(
                        out=rows[:, (2 * hp + 1) * WN : (2 * hp + 2) * WN],
                        in_=el_t[WN:P, WN:P])
                    tp = pst.tile([P, P], DT, tag="tp")
                    nc.tensor.transpose(out=tp[:], in_=el_t[:], identity=ident[:])
                    # pack the two valid transpose quadrants onto matching
                    # node columns: Q[0:64,:] = z_{2hp}^T, Q[64:128,:] = z_{2hp+1}^T
                    q_t = fp.tile([P, WN], DT, tag=f"sT{hp}")
                    nc.vector.tensor_copy(q_t[0:WN, :], tp[0:WN, 0:WN])
                    nc.vector.tensor_copy(q_t[WN:P, :], tp[WN:P, WN:P])
                    sT.append(q_t)
                sc = fp.tile([WN, 16], DT, tag="sc")
                nc.vector.memset(sc[:], 0.0)
                nc.vector.memset(sc[:, 0:1], 1.0)
                sp = pst.tile([WN, 8], DT, tag="acc")
                for hp in range(2):
                    nc.tensor.matmul(sp[:], lhsT=sT[hp][:], rhs=wlr2sb[hp][:],
                                     start=(hp == 0), stop=(hp == 1))
                nc.vector.tensor_copy(sc[:, 1:9], sp[:])
                nc.sync.dma_start(out=rows[:, 256:272], in_=sc[:])

            # ---------------- layer 2 flush ----------------
            def flush2(w, norms):
                po = pst.tile([WN, 256], DT, tag="acc")
                first = True
                for hp in range(2):
                    T = []
                    for i in range(2):
                        tp = pst.tile([P, P], DT, tag="tp")
                        nc.tensor.transpose(out=tp[:],
                                            in_=norms[hp][:, i * P : (i + 1) * P],
                                            identity=ident[:])
                        a = fp.tile([P, P], DT, tag=f"T2_{i}")
                        nc.vector.tensor_copy(a[:], tp[:])
                        T.append(a)
                    for hh in range(2):
                        h = 2 * hp + hh
                        for i in range(2):
                            nc.tensor.matmul(
                                po[:],
                                lhsT=T[i][:, hh * WN : (hh + 1) * WN],
                                rhs=W2sb[i][:, h * 256 : (h + 1) * 256],
                                start=first, stop=False)
                            first = False
                nc.tensor.matmul(po[:], lhsT=ones1[:], rhs=b2sb[:],
                                 start=False, stop=True)
                out = fp.tile([WN, 256], DT, tag="out2")
                nc.vector.tensor_scalar(out=out[:], in0=po[:], scalar1=0.25,
                                        scalar2=None, op0=ALU.mult)
                nc.sync.dma_start(out=logits_d[w * WN : (w + 1) * WN, :], in_=out[:])

            # ---------------- emit the three layers ----------------
            layer(0, ext0_d, FW0, [(0, 129), (0, 129)], 129, 133, 129, flush0)
            if phases >= 2:
                nc.gpsimd.collective_compute(
                    "AllGather", mybir.AluOpType.bypass, replica_groups=RG,
                    ins=[z1_slice[:, :]], outs=[z1_full[:, :]])
                import os as _os
                layer(1, z1_full, FW12, [(0, 129), (129, 258)], 258, 262, 129,
                      flush1_stub if _os.environ.get("GAT_STUB1") else flush1)
            if phases >= 3:
                nc.gpsimd.collective_compute(
                    "AllGather", mybir.AluOpType.bypass, replica_groups=RG,
                    ins=[h2_slice[:, :]], outs=[h2_full[:, :]])
                layer(2, h2_full, FW12, [(0, 257), (0, 257)], 257, 261, 257,
                      flush2)

    nc.compile()
    return nc


def _host_inputs(pp, feat, W0, al0, ar0, b0, W1, al1, ar1, b1, W2, al2, ar2, b2):
    """Build per-core in_maps."""
    NWIN = pp["nwin"]
    NSLOT = pp["nslot"]
    KP = pp["kp"]
    NWIN_C = NWIN // NCORE
    slot2node = pp["slot2node"]

    def score_w(W, al, ar, hw):
        Wr = W.reshape(W.shape[0], HEADS, hw)
        wl = np.einsum("khf,hf->kh", Wr, al).astype(np.float32)
        wr = np.einsum("khf,hf->kh", Wr, ar).astype(np.float32)
        return wl, wr

    wl0, wr0 = score_w(W0, al0, ar0, HID)
    wl1, wr1 = score_w(W1, al1, ar1, HID)
    wl2, wr2 = score_w(W2, al2, ar2, N_CLASSES)

    feat_s = np.zeros((NSLOT, IN_F), np.float32)
    valid = slot2node >= 0
    feat_s[valid] = feat[slot2node[valid]]
    ext0 = np.zeros((NSLOT, FW0), np.float32)
    ext0[:, 0:128] = feat_s
    ext0[:, 128] = 1.0
    ext0[:, 129:133] = feat_s @ wl0
    ext0[:, 133:137] = feat_s @ wr0

    wlr1 = np.concatenate([wl1, wr1], axis=1).astype(np.float32)
    wlr2 = np.concatenate([wl2, wr2], axis=1).astype(np.float32)
    b1s = np.zeros((2, P, P), np.float32)
    b1r = b1.reshape(HEADS, HID)
    for hp in range(2):
        b1s[hp, 0:WN, 0:WN] = b1r[2 * hp][None, :]
        b1s[hp, WN:P, WN:P] = b1r[2 * hp + 1][None, :]
    b2m = b2.reshape(HEADS, N_CLASSES).mean(axis=0).astype(np.float32)

    W0big = np.zeros((HEADS, IN_F, 256), np.float32)
    for h in range(HEADS):
        W0big[h, :, h * HID : (h + 1) * HID] = W0[:, h * HID : (h + 1) * HID]
    common = {
        "ext0": ext0,
        "W0big": W0big,
        "W1": W1.astype(np.float32),
        "W2": W2.astype(np.float32),
        "wlr1": wlr1, "wlr2": wlr2,
        "b0row": b0.reshape(1, 256).astype(np.float32),
        "b1s": b1s,
        "b2mean": b2m.reshape(1, 256),
    }
    in_maps = []
    for c in range(NCORE):
        ws = slice(c * NWIN_C, (c + 1) * NWIN_C)
        wi = np.concatenate([pp["srcA"][ws], pp["dstB"][ws]], axis=2)
        in_maps.append({**common,
                        "windat_i": np.ascontiguousarray(wi),
                        "windat_f": np.ascontiguousarray(pp["dloc"][ws])})
    return in_maps


def _install_ntff_hook_shim():
    """Provide antenv.axon_hooks (NTFF profiling) if the image lacks it."""
    import sys, types, contextlib, ctypes, os
    if "antenv.axon_hooks" in sys.modules:
        return
    try:
        from antenv import axon_hooks  # noqa: F401
        return
    except ImportError:
        pass
    so_path = "/opt/axon/libaxon_pjrt.so"
    mod = types.ModuleType("antenv.axon_hooks")
    state = {"hook": None}

    def _make_hook():
        if not os.path.exists(so_path):
            return None
        lib = ctypes.CDLL(so_path)
        if not hasattr(lib, "axon_start_nrt_profile"):
            return None
        lib.axon_start_nrt_profile.argtypes = [
            ctypes.POINTER(ctypes.c_int64), ctypes.c_size_t]
        lib.axon_start_nrt_profile.restype = ctypes.c_int64
        lib.axon_stop_nrt_profile.argtypes = [ctypes.c_char_p]
        lib.axon_stop_nrt_profile.restype = ctypes.c_int64

        @contextlib.contextmanager
        def _hook(output_dir, device_ids):
            import jax
            jax.devices()
            if device_ids:
                ids = (ctypes.c_int64 * len(device_ids))(*device_ids)
                rc = lib.axon_start_nrt_profile(ids, len(device_ids))
            else:
                rc = lib.axon_start_nrt_profile(None, 0)
            if rc != 0:
                raise RuntimeError(f"axon_start_nrt_profile rc={rc}")
            try:
                yield
            finally:
                n = lib.axon_stop_nrt_profile(str(output_dir).encode())
                print(f"profile: {n} file(s) -> {output_dir}", file=sys.stderr)

        return _hook

    def get_axon_ntff_profile_hook():
        if state["hook"] is None:
            state["hook"] = _make_hook()
        return state["hook"]

    mod.get_axon_ntff_profile_hook = get_axon_ntff_profile_hook
    mod.set_axon_ntff_profile_hook = lambda h: state.update(hook=h)
    sys.modules["antenv.axon_hooks"] = mod


def kernel(feat, W0, al0, ar0, b0, W1, al1, ar1, b1, W2, al2, ar2, b2, src, dst,
           trace=False):
    from concourse import bass_utils
    if trace:
        _install_ntff_hook_shim()

    feat = np.asarray(feat); src = np.asarray(src); dst = np.asarray(dst)
    key = (src.tobytes()[:64], dst.tobytes()[:64], len(src))
    if key not in _CACHE:
        pp = _preprocess(src, dst, feat.shape[0])
        nc = _build(pp)
        _CACHE[key] = (pp, nc)
    pp, nc = _CACHE[key]

    in_maps = _host_inputs(pp, feat,
                           np.asarray(W0), np.asarray(al0), np.asarray(ar0), np.asarray(b0),
                           np.asarray(W1), np.asarray(al1), np.asarray(ar1), np.asarray(b1),
                           np.asarray(W2), np.asarray(al2), np.asarray(ar2), np.asarray(b2))
    res = bass_utils.run_bass_kernel_spmd(
        nc, in_maps, core_ids=list(range(NCORE)), trace=trace)
    if trace:
        kernel.last_exec_ns = res.exec_time_ns
        kernel.last_trace = res.instructions_and_trace

    logits_s = np.concatenate([res.results[c]["logits"] for c in range(NCORE)])
    out = logits_s[pp["slot_of"][np.arange(feat.shape[0])]]
    return np.ascontiguousarray(out.astype(np.float32))
